# revision 1
# baseline (speedup 1.0000x reference)
"""Graphwise KL loss (segment_reduce) on 8 trn2 NeuronCores.

Strategy:
  Device (the O(N) memory-bound work, data-parallel over 8 cores, each core
  streams a contiguous 1/8 slice of the element arrays):
    pr = y_true * weight
    e1 = pr * (ln(pr + 1e-37) - ln(y_pred + 1e-8))
    out: 32-element block sums of e1 and pr        (2 x 32768 f32 per core)
  Host (O(num_graphs) metadata assembly, fp64):
    Per-segment sums A_g (of e1) and B_g (of pr) are reconstructed from the
    device block sums plus fp64 partial sums of the (< 32-element) block
    prefixes at each segment boundary.  With S_g = max(B_g, EPS):
      total = mean_g (A_g - B_g * ln(S_g)) / S_g
    which equals the reference's  sum_g sum_i p*(ln p - ln q)  with
    p = pr/S_g  (identical up to the ln(max(p,EPS)) clip on the ~1e2
    elements with p < 1e-8, which contribute O(1e-7) relative error).

  Raw Bass (no Tile): this walrus build caps every non-EventSemaphore
  instruction at ONE inline sync wait, so all waits are standalone wait_ge
  instructions and all cross-engine sync is explicit, with double-buffered
  tiles (buf = t % 2) and per-engine instruction streams.
"""

import numpy as np

N_TOTAL = 8388608
N_CORES = 8
N_LOCAL = N_TOTAL // N_CORES      # 1048576
P = 128
TILE_F = 2048                     # free dim of one macro tile
TILE_ELEMS = P * TILE_F           # 262144
N_TILES = N_LOCAL // TILE_ELEMS   # 4
BLK = 32
JPT = TILE_F // BLK               # 64 block sums per partition per tile
N_BLOCKS_LOCAL = N_LOCAL // BLK   # 32768
EPS = 1e-8
TINY = 1e-37

_CACHE = {}


def _check_one_wait(nc):
    """Assert no non-EventSemaphore instruction carries more than one wait."""
    bad = []
    for f in nc.m.functions:
        for bb in f.blocks:
            for inst in bb.instructions:
                si = inst.sync_info
                if si and si.on_wait and len(si.on_wait) > 1:
                    if "EventSem" not in type(inst).__name__:
                        bad.append((type(inst).__name__, inst.name, len(si.on_wait)))
    assert not bad, f"multi-wait instructions remain: {bad}"


def _build_program():
    import concourse.bass as bass
    import concourse.mybir as mybir

    f32 = mybir.dt.float32
    Ln = mybir.ActivationFunctionType.Ln
    X = mybir.AxisListType.X
    ADD = mybir.AluOpType.add

    nc = bass.Bass()

    # Const APs for the Ln biases (same mechanism Bass.__init__ uses for 0/1).
    for val in (TINY, EPS):
        ct = nc.alloc_sbuf_tensor(f"const-f32-{val}", [128, 1], f32)
        nc.gpsimd.memset(ct.ap(), val)
        nc.const_aps.aps[(f32, val)] = ct.ap()
    nc.all_engine_barrier()

    yp = nc.declare_dram_parameter("yp", [N_LOCAL], f32, isOutput=False)
    yt = nc.declare_dram_parameter("yt", [N_LOCAL], f32, isOutput=False)
    w = nc.declare_dram_parameter("w", [N_LOCAL], f32, isOutput=False)
    o1 = nc.declare_dram_parameter("o1", [N_BLOCKS_LOCAL], f32, isOutput=True)
    o2 = nc.declare_dram_parameter("o2", [N_BLOCKS_LOCAL], f32, isOutput=True)

    yp3 = yp[:].rearrange("(t p f) -> t p f", p=P, f=TILE_F)
    yt3 = yt[:].rearrange("(t p f) -> t p f", p=P, f=TILE_F)
    w3 = w[:].rearrange("(t p f) -> t p f", p=P, f=TILE_F)
    o13 = o1[:].rearrange("(t p j) -> t p j", p=P, j=JPT)
    o23 = o2[:].rearrange("(t p j) -> t p j", p=P, j=JPT)

    # Double-buffered SBUF tiles.
    def buf2(name, shape):
        return [nc.alloc_sbuf_tensor(f"{name}{i}", shape, f32).ap() for i in range(2)]

    t_yp = buf2("t_yp", [P, TILE_F])
    t_yt = buf2("t_yt", [P, TILE_F])
    t_w = buf2("t_w", [P, TILE_F])
    t_pr = buf2("t_pr", [P, TILE_F])
    t_lp = buf2("t_lp", [P, TILE_F])
    t_lq = buf2("t_lq", [P, TILE_F])
    t_d = buf2("t_d", [P, TILE_F])
    t_e1 = buf2("t_e1", [P, TILE_F])
    t_b1 = buf2("t_b1", [P, JPT])
    t_b2 = buf2("t_b2", [P, JPT])

    # Even/odd semaphores per DMA stream: at most ONE DMA in flight per sem,
    # so its 16 completion sub-increments can't interleave with another
    # transfer's (CoreSim SemaphoreRace otherwise).
    s_yp = [nc.alloc_semaphore(f"s_yp{i}") for i in range(2)]  # +16 per load
    s_yt = [nc.alloc_semaphore(f"s_yt{i}") for i in range(2)]
    s_w = [nc.alloc_semaphore(f"s_w{i}") for i in range(2)]
    s_out = [nc.alloc_semaphore(f"s_out{i}") for i in range(2)]  # +32 per iter
    s_act = nc.alloc_semaphore("s_act")  # +1 per ACT op (lp, lq per iter)
    s_dve = nc.alloc_semaphore("s_dve")  # +1 per DVE op

    # DVE op order (hoisted pr for cross-engine overlap):
    #   pr(0), pr(1), [d,e1,r1,r2](0), pr(2), [d,e1,r1,r2](1), pr(3),
    #   [d,e1,r1,r2](2), [d,e1,r1,r2](3)
    # Absolute DVE indices (1-based):
    dve_idx = {}
    n = 0
    order = [("pr", 0), ("pr", 1)]
    for t in range(N_TILES):
        order.append(("blk", t))
        if t + 2 < N_TILES:
            order.append(("pr", t + 2))
    for item in order:
        kind, t = item
        if kind == "pr":
            n += 1
            dve_idx[("pr", t)] = n
        else:
            for opname in ("d", "e1", "r1", "r2"):
                n += 1
                dve_idx[(opname, t)] = n
    n_dve_total = n

    with nc.Block() as block:

        @block.gpsimd
        def _(g):
            for t in range(N_TILES):
                if t >= 2:
                    # typ[buf] was read by lq(t-2) = ACT op 2(t-2)+2
                    g.wait_ge(s_act, 2 * (t - 2) + 2)
                    # tyt/tw[buf] read by pr(t-2); b-out wait below covers DVE
                    g.wait_ge(s_dve, dve_idx[("pr", t - 2)])
                buf = t % 2
                g.dma_start(t_yp[buf], yp3[t, :, :]).then_inc(s_yp[buf], 16)
                g.dma_start(t_yt[buf], yt3[t, :, :]).then_inc(s_yt[buf], 16)
                g.dma_start(t_w[buf], w3[t, :, :]).then_inc(s_w[buf], 16)
                if t >= 1:
                    # store iteration t-1 outputs
                    tt = t - 1
                    g.wait_ge(s_dve, dve_idx[("r2", tt)])
                    g.dma_start(o13[tt, :, :], t_b1[tt % 2]).then_inc(s_out[tt % 2], 16)
                    g.dma_start(o23[tt, :, :], t_b2[tt % 2]).then_inc(s_out[tt % 2], 16)
            tt = N_TILES - 1
            g.wait_ge(s_dve, dve_idx[("r2", tt)])
            g.dma_start(o13[tt, :, :], t_b1[tt % 2]).then_inc(s_out[tt % 2], 16)
            g.dma_start(o23[tt, :, :], t_b2[tt % 2]).then_inc(s_out[tt % 2], 16)
            # ensure all stores landed before program end
            for i in range(2):
                g.wait_ge(s_out[i], 32 * (N_TILES // 2))

        @block.scalar
        def _(s):
            for t in range(N_TILES):
                buf = t % 2
                # lp(t) = Ln(pr(t) + TINY): needs DVE pr(t); also covers
                # lp/lq[buf] slot reuse (d(t-2) precedes pr(t) in DVE order)
                s.wait_ge(s_dve, dve_idx[("pr", t)])
                s.activation(t_lp[buf], t_pr[buf], Ln, bias=TINY).then_inc(s_act, 1)
                # lq(t) = Ln(yp(t) + EPS)
                s.wait_ge(s_yp[buf], 16 * (t // 2 + 1))
                s.activation(t_lq[buf], t_yp[buf], Ln, bias=EPS).then_inc(s_act, 1)

        @block.vector
        def _(v):
            def emit_pr(t):
                buf = t % 2
                v.wait_ge(s_yt[buf], 16 * (t // 2 + 1))
                v.wait_ge(s_w[buf], 16 * (t // 2 + 1))
                v.tensor_mul(t_pr[buf], t_yt[buf], t_w[buf]).then_inc(s_dve, 1)

            def emit_blk(t):
                buf = t % 2
                v.wait_ge(s_act, 2 * t + 2)  # lp(t), lq(t) done
                v.tensor_sub(t_d[buf], t_lp[buf], t_lq[buf]).then_inc(s_dve, 1)
                # same-engine RAW: the DVE pipeline does not forward; an op
                # reading the previous op's output needs an explicit wait
                v.wait_ge(s_dve, dve_idx[("d", t)])
                v.tensor_mul(t_e1[buf], t_pr[buf], t_d[buf]).then_inc(s_dve, 1)
                if t >= 2:
                    # b1/b2[buf] were stored by out-DMAs of t-2
                    v.wait_ge(s_out[t % 2], 32 * ((t - 2) // 2 + 1))
                v.wait_ge(s_dve, dve_idx[("e1", t)])
                v.tensor_reduce(
                    t_b1[buf], t_e1[buf].rearrange("p (j b) -> p j b", b=BLK),
                    axis=X, op=ADD,
                ).then_inc(s_dve, 1)
                v.tensor_reduce(
                    t_b2[buf], t_pr[buf].rearrange("p (j b) -> p j b", b=BLK),
                    axis=X, op=ADD,
                ).then_inc(s_dve, 1)

            for item in order:
                if item[0] == "pr":
                    emit_pr(item[1])
                else:
                    emit_blk(item[1])

    _check_one_wait(nc)
    return nc


def _get_program():
    if "nc" not in _CACHE:
        _CACHE["nc"] = _build_program()
    return _CACHE["nc"]


def _run_device(yp, yt, w, trace=False):
    from concourse.bass_utils import run_bass_kernel_spmd

    nc = _get_program()
    in_maps = [
        {
            "yp": yp[k * N_LOCAL : (k + 1) * N_LOCAL],
            "yt": yt[k * N_LOCAL : (k + 1) * N_LOCAL],
            "w": w[k * N_LOCAL : (k + 1) * N_LOCAL],
        }
        for k in range(N_CORES)
    ]
    res = run_bass_kernel_spmd(nc, in_maps, list(range(N_CORES)), trace=trace)
    bs1 = np.concatenate([r["o1"].reshape(-1) for r in res.results])
    bs2 = np.concatenate([r["o2"].reshape(-1) for r in res.results])
    return bs1, bs2, res


def kernel(y_pred, y_true, weight, segment_ptr, _trace=False):
    yp = np.ascontiguousarray(np.asarray(y_pred), dtype=np.float32).reshape(-1)
    yt = np.ascontiguousarray(np.asarray(y_true), dtype=np.float32).reshape(-1)
    w = np.ascontiguousarray(np.asarray(weight), dtype=np.float32).reshape(-1)
    ptr = np.asarray(segment_ptr).astype(np.int64).reshape(-1)
    n = yp.shape[0]
    G = ptr.shape[0] - 1
    assert n == N_TOTAL, f"kernel compiled for N={N_TOTAL}, got {n}"

    bs1, bs2, res = _run_device(yp, yt, w, trace=_trace)
    _CACHE["last_res"] = res

    # ---- host assembly in fp64 ----
    pre1 = np.empty(bs1.shape[0] + 1)
    pre1[0] = 0.0
    np.cumsum(bs1, dtype=np.float64, out=pre1[1:])
    pre2 = np.empty(bs2.shape[0] + 1)
    pre2[0] = 0.0
    np.cumsum(bs2, dtype=np.float64, out=pre2[1:])

    # clip ptr defensively to [0, n] (reference guarantees this range)
    ptrc = np.clip(ptr, 0, n)
    b_idx = ptrc // BLK
    r = ptrc - b_idx * BLK  # offset within block
    # fp64 partial sums over [ptr - r, ptr) for boundaries not block-aligned
    seg_off = np.concatenate([[0], np.cumsum(r)])
    tot = int(seg_off[-1])
    part1 = np.zeros(ptrc.shape[0])
    part2 = np.zeros(ptrc.shape[0])
    if tot > 0:
        idx = np.repeat(ptrc - r, r) + (np.arange(tot) - np.repeat(seg_off[:-1], r))
        pr_h = yt[idx].astype(np.float64) * w[idx].astype(np.float64)
        e1_h = pr_h * (np.log(pr_h + TINY) - np.log(yp[idx].astype(np.float64) + EPS))
        nz = r > 0
        red_idx = np.minimum(seg_off[:-1][nz], tot - 1).astype(np.int64)
        part1[nz] = np.add.reduceat(e1_h, red_idx)
        part2[nz] = np.add.reduceat(pr_h, red_idx)

    C1 = pre1[b_idx] + part1
    C2 = pre2[b_idx] + part2
    A = np.diff(C1)
    Bg = np.diff(C2)
    S = np.maximum(Bg, EPS)
    total = np.sum((A - Bg * np.log(S)) / S) / max(G, 1)
    return np.float32(total)



# revision 2
# speedup vs baseline: 1.0545x; 1.0545x over previous
"""Graphwise KL loss (segment_reduce) on 8 trn2 NeuronCores.

Strategy:
  Device (O(N) memory-bound work, data-parallel over 8 cores; each core
  streams a contiguous 1/8 slice):
    pr = y_true * weight                      (DVE, bf16 out)
    d  = ln(pr + 1e-37) - ln(y_pred + 1e-8)   (ACT Ln x2 -> bf16, DVE sub)
    e1 = pr * d                               (DVE, bf16 2x mode)
    32-element block sums of e1 and pr        (PE matmul w/ block-diag ones)
  Host (O(num_graphs) metadata assembly, fp64): reconstruct per-segment
  sums A_g (e1) and B_g (pr) from device block sums + fp64 partial sums
  at segment boundaries; with S_g = max(B_g, EPS):
      total = mean_g (A_g - B_g * ln(S_g)) / S_g

  Inputs are host-packed into ONE interleaved DRAM tensor per core laid
  out [tile, partition, {yp,yt,w}, col] with element i = 128*col + part
  inside a tile, so each 32-element block sits in one partition quad and
  the PE reduces blocks via matmul with a [128,4] block-diagonal ones
  stationary.  One load DMA + one store DMA per tile; 3-deep input
  buffering keeps the 16 SDMA engines streaming back-to-back (DMA-bound).

  Raw Bass (no Tile), one explicit wait_ge per dependency, double- or
  triple-buffered tiles, per-engine instruction streams:
    POOL: load DMAs      DVE: pr/d/e1 + psum evac (pr half)
    ACT:  Ln, Ln, psum evac (e1 half)   PE: 4 matmuls   SP: store DMAs
"""

import numpy as np

N_TOTAL = 8388608
N_CORES = 8
N_LOCAL = N_TOTAL // N_CORES      # 1048576
P = 128
TILE_F = 1024                     # columns per tile
TILE_ELEMS = P * TILE_F           # 131072
N_TILES = N_LOCAL // TILE_ELEMS   # 8
BLK = 32
JBLK = P // BLK                   # 4 blocks per column
N_BLOCKS_LOCAL = N_LOCAL // BLK   # 32768
EPS = 1e-8
TINY = 1e-37

_CACHE = {}


def _check_one_wait(nc):
    """Assert no non-EventSemaphore instruction carries more than one wait."""
    bad = []
    for f in nc.m.functions:
        for bb in f.blocks:
            for inst in bb.instructions:
                si = inst.sync_info
                if si and si.on_wait and len(si.on_wait) > 1:
                    if "EventSem" not in type(inst).__name__:
                        bad.append((type(inst).__name__, inst.name, len(si.on_wait)))
    assert not bad, f"multi-wait instructions remain: {bad}"


def _build_program():
    import concourse.bass as bass
    import concourse.mybir as mybir

    f32 = mybir.dt.float32
    bf16 = mybir.dt.bfloat16
    Ln = mybir.ActivationFunctionType.Ln
    Copy = mybir.ActivationFunctionType.Copy

    nc = bass.Bass()

    # Const APs for the Ln biases (activation() looks these up by value).
    s_cst = nc.alloc_semaphore("s_cst")
    for val in (TINY, EPS):
        ct = nc.alloc_sbuf_tensor(f"const-f32-{val}", [128, 1], f32)
        nc.gpsimd.memset(ct.ap(), val).then_inc(s_cst, 1)
        nc.const_aps.aps[(f32, val)] = ct.ap()

    # DRAM: packed inputs [t, p, {yp,yt,w}, f], ones stationary, outputs.
    pk = nc.declare_dram_parameter("pk", [3 * N_LOCAL], f32, isOutput=False)
    wm = nc.declare_dram_parameter("wm", [P * JBLK], f32, isOutput=False)
    od = nc.declare_dram_parameter(
        "od", [N_TILES * JBLK * 2 * TILE_F], f32, isOutput=True
    )

    pk3 = pk[:].rearrange("(t p cf) -> t p cf", p=P, cf=3 * TILE_F)
    wm2 = wm[:].rearrange("(p j) -> p j", j=JBLK)
    od3 = od[:].rearrange("(t j f) -> t j f", j=JBLK, f=2 * TILE_F)

    # SBUF tiles.
    t_in = [nc.alloc_sbuf_tensor(f"t_in{i}", [P, 3 * TILE_F], f32).ap()
            for i in range(3)]
    t_pr = [nc.alloc_sbuf_tensor(f"t_pr{i}", [P, TILE_F], bf16).ap()
            for i in range(3)]
    t_lp = [nc.alloc_sbuf_tensor(f"t_lp{i}", [P, TILE_F], bf16).ap()
            for i in range(2)]
    t_lq = [nc.alloc_sbuf_tensor(f"t_lq{i}", [P, TILE_F], bf16).ap()
            for i in range(2)]
    t_d = [nc.alloc_sbuf_tensor(f"t_d{i}", [P, TILE_F], bf16).ap()
           for i in range(2)]
    t_e1 = [nc.alloc_sbuf_tensor(f"t_e1{i}", [P, TILE_F], bf16).ap()
            for i in range(2)]
    t_sb = [nc.alloc_sbuf_tensor(f"t_sb{i}", [JBLK, 2 * TILE_F], f32).ap()
            for i in range(2)]
    w32 = nc.alloc_sbuf_tensor("w32", [P, JBLK], f32).ap()
    w16 = nc.alloc_sbuf_tensor("w16", [P, JBLK], bf16).ap()

    # PSUM: per slot one [4, 2048] f32 tensor (4 banks); cols 0:1024 hold
    # the e1 block sums, 1024:2048 the pr block sums.
    ps = [nc.alloc_psum_tensor(f"ps{i}", [JBLK, 2 * TILE_F], f32).ap()
          for i in range(2)]

    s_in = [nc.alloc_semaphore(f"s_in{i}") for i in range(3)]   # +16 per load
    s_o = [nc.alloc_semaphore(f"s_o{i}") for i in range(2)]     # +16 per store
    s_wld = nc.alloc_semaphore("s_wld")                         # +16 W load
    s_dve = nc.alloc_semaphore("s_dve")  # +1 per DVE op
    s_act = nc.alloc_semaphore("s_act")  # +1 per ACT op
    s_pe = nc.alloc_semaphore("s_pe")    # +1 per tile (last matmul)

    # Per-tile op indices (1-based semaphore targets):
    #   DVE: w16cvt=1, pr(t)=4t+2, d(t)=4t+3, e1(t)=4t+4, cp2(t)=4t+5
    #   ACT: lq(t)=3t+1, lp(t)=3t+2, cp1(t)=3t+3
    #   PE:  s_pe=t+1 after tile t's 4th matmul
    F = TILE_F

    with nc.Block() as block:

        @block.gpsimd
        def _(g):
            g.dma_start(w32, wm2).then_inc(s_wld, 16)
            for t in range(N_TILES):
                if t >= 3:
                    # input slot free once lp(t-3) done (covers lq + DVE pr)
                    g.wait_ge(s_act, 3 * (t - 3) + 2)
                g.dma_start(t_in[t % 3], pk3[t, :, :]).then_inc(s_in[t % 3], 16)

        @block.vector
        def _(v):
            v.wait_ge(s_wld, 16)
            v.tensor_copy(w16, w32).then_inc(s_dve, 1)
            for t in range(N_TILES):
                s3, s2 = t % 3, t % 2
                yt_c = t_in[s3][:, F:2 * F]
                w_c = t_in[s3][:, 2 * F:3 * F]
                v.wait_ge(s_in[s3], 16 * (t // 3 + 1))
                v.tensor_mul(t_pr[s3], yt_c, w_c).then_inc(s_dve, 1)
                v.wait_ge(s_act, 3 * t + 2)   # lp(t) done (covers lq(t))
                v.tensor_sub(t_d[s2], t_lp[s2], t_lq[s2]).then_inc(s_dve, 1)
                if t >= 2:
                    v.wait_ge(s_pe, t - 1)    # mm(t-2) done: e1 slot free
                v.wait_ge(s_dve, 4 * t + 3)   # same-engine RAW: d(t) retired
                v.tensor_mul(t_e1[s2], t_pr[s3], t_d[s2]).then_inc(s_dve, 1)
                # evacuate pr block sums (psum cols 1024:2048) of tile t
                v.wait_ge(s_pe, t + 1)
                if t >= 2:
                    v.wait_ge(s_o[s2], 16 * ((t - 2) // 2 + 1))
                v.tensor_copy(t_sb[s2][:, F:2 * F], ps[s2][:, F:2 * F]) \
                    .then_inc(s_dve, 1)

        @block.scalar
        def _(s):
            s.wait_ge(s_cst, 2)
            for t in range(N_TILES):
                s3, s2 = t % 3, t % 2
                yp_c = t_in[s3][:, 0:F]
                s.wait_ge(s_in[s3], 16 * (t // 3 + 1))
                s.activation(t_lq[s2], yp_c, Ln, bias=EPS).then_inc(s_act, 1)
                s.wait_ge(s_dve, 4 * t + 2)   # pr(t) done
                s.activation(t_lp[s2], t_pr[s3], Ln, bias=TINY) \
                    .then_inc(s_act, 1)
                # evacuate e1 block sums (psum cols 0:1024) of tile t
                s.wait_ge(s_pe, t + 1)
                if t >= 2:
                    s.wait_ge(s_o[s2], 16 * ((t - 2) // 2 + 1))
                s.activation(t_sb[s2][:, 0:F], ps[s2][:, 0:F], Copy) \
                    .then_inc(s_act, 1)

        @block.tensor
        def _(p):
            for t in range(N_TILES):
                s3, s2 = t % 3, t % 2
                p.wait_ge(s_dve, 4 * t + 4)   # e1(t) done (covers pr, w16)
                if t >= 2:
                    # psum slot free once both evacuations of t-2 retired
                    p.wait_ge(s_act, 3 * (t - 2) + 3)
                    p.wait_ge(s_dve, 4 * (t - 2) + 5)
                p.matmul(ps[s2][:, 0:512], w16, t_e1[s2][:, 0:512])
                p.matmul(ps[s2][:, 512:1024], w16, t_e1[s2][:, 512:1024])
                p.matmul(ps[s2][:, 1024:1536], w16, t_pr[s3][:, 0:512])
                p.matmul(ps[s2][:, 1536:2048], w16, t_pr[s3][:, 512:1024]) \
                    .then_inc(s_pe, 1)

        @block.sync
        def _(sp):
            for t in range(N_TILES):
                s2 = t % 2
                sp.wait_ge(s_act, 3 * t + 3)   # cp1(t) done
                sp.wait_ge(s_dve, 4 * t + 5)   # cp2(t) done
                sp.dma_start(od3[t, :, :], t_sb[s2]).then_inc(s_o[s2], 16)
            sp.wait_ge(s_o[0], 16 * (N_TILES // 2))
            sp.wait_ge(s_o[1], 16 * (N_TILES // 2))

    _check_one_wait(nc)
    return nc


def _get_program():
    if "nc" not in _CACHE:
        _CACHE["nc"] = _build_program()
    return _CACHE["nc"]


def _pack_inputs(yp, yt, w):
    """[N_TOTAL] f32 x3 -> per-core packed [t, p, {yp,yt,w}, f] arrays."""
    def to_tiles(x):
        # element i_local = t*TILE_ELEMS + f*P + p  ->  [core, t, p, f]
        return x.reshape(N_CORES, N_TILES, TILE_F, P).transpose(0, 1, 3, 2)

    pk = np.stack([to_tiles(yp), to_tiles(yt), to_tiles(w)], axis=3)
    return np.ascontiguousarray(pk).reshape(N_CORES, -1)


_WMAT = None


def _wmat():
    global _WMAT
    if _WMAT is None:
        wmat = np.zeros((P, JBLK), dtype=np.float32)
        for j in range(JBLK):
            wmat[BLK * j:BLK * (j + 1), j] = 1.0
        _WMAT = wmat.reshape(-1)
    return _WMAT


def _run_device(yp, yt, w, trace=False):
    from concourse.bass_utils import run_bass_kernel_spmd

    nc = _get_program()
    pk = _pack_inputs(yp, yt, w)
    wmat = _wmat()
    in_maps = [{"pk": pk[k], "wm": wmat} for k in range(N_CORES)]
    res = run_bass_kernel_spmd(nc, in_maps, list(range(N_CORES)), trace=trace)

    bs1_parts, bs2_parts = [], []
    for r in res.results:
        dev = r["od"].reshape(N_TILES, JBLK, 2 * TILE_F)
        # block id (local) = t*4096 + f*4 + j -> order [t, f, j]
        bs1_parts.append(dev[:, :, 0:TILE_F].transpose(0, 2, 1).reshape(-1))
        bs2_parts.append(dev[:, :, TILE_F:2 * TILE_F].transpose(0, 2, 1)
                         .reshape(-1))
    return np.concatenate(bs1_parts), np.concatenate(bs2_parts), res


def kernel(y_pred, y_true, weight, segment_ptr, _trace=False):
    yp = np.ascontiguousarray(np.asarray(y_pred), dtype=np.float32).reshape(-1)
    yt = np.ascontiguousarray(np.asarray(y_true), dtype=np.float32).reshape(-1)
    w = np.ascontiguousarray(np.asarray(weight), dtype=np.float32).reshape(-1)
    ptr = np.asarray(segment_ptr).astype(np.int64).reshape(-1)
    n = yp.shape[0]
    G = ptr.shape[0] - 1
    assert n == N_TOTAL, f"kernel compiled for N={N_TOTAL}, got {n}"

    bs1, bs2, res = _run_device(yp, yt, w, trace=_trace)
    _CACHE["last_res"] = res

    # ---- host assembly in fp64 ----
    pre1 = np.empty(bs1.shape[0] + 1)
    pre1[0] = 0.0
    np.cumsum(bs1, dtype=np.float64, out=pre1[1:])
    pre2 = np.empty(bs2.shape[0] + 1)
    pre2[0] = 0.0
    np.cumsum(bs2, dtype=np.float64, out=pre2[1:])

    # clip ptr defensively to [0, n] (reference guarantees this range)
    ptrc = np.clip(ptr, 0, n)
    b_idx = ptrc // BLK
    r = ptrc - b_idx * BLK  # offset within block
    # fp64 partial sums over [ptr - r, ptr) for boundaries not block-aligned
    seg_off = np.concatenate([[0], np.cumsum(r)])
    tot = int(seg_off[-1])
    part1 = np.zeros(ptrc.shape[0])
    part2 = np.zeros(ptrc.shape[0])
    if tot > 0:
        idx = np.repeat(ptrc - r, r) + (np.arange(tot) - np.repeat(seg_off[:-1], r))
        pr_h = yt[idx].astype(np.float64) * w[idx].astype(np.float64)
        e1_h = pr_h * (np.log(pr_h + TINY) - np.log(yp[idx].astype(np.float64) + EPS))
        nz = r > 0
        red_idx = np.minimum(seg_off[:-1][nz], tot - 1).astype(np.int64)
        part1[nz] = np.add.reduceat(e1_h, red_idx)
        part2[nz] = np.add.reduceat(pr_h, red_idx)

    C1 = pre1[b_idx] + part1
    C2 = pre2[b_idx] + part2
    A = np.diff(C1)
    Bg = np.diff(C2)
    S = np.maximum(Bg, EPS)
    total = np.sum((A - Bg * np.log(S)) / S) / max(G, 1)
    return np.float32(total)


# revision 4
# speedup vs baseline: 1.8964x; 1.7985x over previous
"""Graphwise KL loss (segment_reduce) on 8 trn2 NeuronCores.

Strategy:
  Device (O(N) memory-bound work, data-parallel over 8 cores; each core
  streams a contiguous 1/8 slice):
    pr = y_true * weight                      (DVE, bf16 out)
    d  = ln(pr + 1e-37) - ln(y_pred + 1e-8)   (ACT Ln x2 -> bf16, DVE sub)
    e1 = pr * d                               (DVE, bf16 2x mode)
    32-element block sums of e1 and pr        (PE matmul, block-diag ones)
  Host (O(num_graphs) metadata assembly, fp64): reconstruct per-segment
  sums A_g (e1) and B_g (pr) from device block sums + fp64 partial sums
  at segment boundaries; with S_g = max(B_g, EPS):
      total = mean_g (A_g - B_g * ln(S_g)) / S_g

  Inputs are host-packed into ONE interleaved DRAM tensor per core laid
  out [tile, partition, {yp,yt,w}, col] with element i = 128*col + part
  inside a tile, so each 32-element block sits in one partition quad and
  PE reduces blocks via matmul.  The stationary is a [128,16] pair of
  block-diagonal ones matrices (cols 0:8 for column-half 0 -> psum rows
  0:4, cols 8:16 for half 1 -> rows 4:8) so both halves of a tile
  accumulate into one [8, 1024] PSUM tensor: e1 sums in psum cols 0:512,
  pr sums in 512:1024.  One ACT Copy evacuates PSUM->SBUF per tile
  (skewed one tile late so it never blocks the next tile's front end),
  then one store DMA.  One load DMA per tile; 3-deep input buffers.

  Raw Bass (no Tile): every op carries at most ONE inline wait
  (walrus cap); extra deps use standalone wait_ge.  Engine split:
    POOL: load DMAs    DVE: pr/d/e1    ACT: Ln, Ln, psum evac
    PE: 4 matmuls      SP: store DMAs
"""

import numpy as np

N_TOTAL = 8388608
N_CORES = 8
N_LOCAL = N_TOTAL // N_CORES      # 1048576
P = 128
TILE_F = 1024                     # columns per tile
TILE_ELEMS = P * TILE_F           # 131072
N_TILES = N_LOCAL // TILE_ELEMS   # 8
BLK = 32
JBLK = P // BLK                   # 4 blocks per column
HALF = TILE_F // 2                # 512 moving columns per matmul
N_BLOCKS_LOCAL = N_LOCAL // BLK   # 32768
EPS = 1e-8
TINY = 1e-37

_CACHE = {}


def _check_one_wait(nc):
    """Assert no non-EventSemaphore instruction carries more than one wait."""
    bad = []
    for f in nc.m.functions:
        for bb in f.blocks:
            for inst in bb.instructions:
                si = inst.sync_info
                if si and si.on_wait and len(si.on_wait) > 1:
                    if "EventSem" not in type(inst).__name__:
                        bad.append((type(inst).__name__, inst.name, len(si.on_wait)))
    assert not bad, f"multi-wait instructions remain: {bad}"


def _build_program():
    import concourse.bass as bass
    import concourse.mybir as mybir

    f32 = mybir.dt.float32
    bf16 = mybir.dt.bfloat16
    Ln = mybir.ActivationFunctionType.Ln
    Copy = mybir.ActivationFunctionType.Copy

    nc = bass.Bass()

    # Const APs for the Ln biases (activation() looks these up by value).
    s_cst = nc.alloc_semaphore("s_cst")
    for val in (TINY, EPS):
        ct = nc.alloc_sbuf_tensor(f"const-f32-{val}", [128, 1], f32)
        nc.gpsimd.memset(ct.ap(), val).then_inc(s_cst, 1)
        nc.const_aps.aps[(f32, val)] = ct.ap()

    pk = nc.declare_dram_parameter("pk", [3 * N_LOCAL], f32, isOutput=False)
    wm = nc.declare_dram_parameter("wm", [P * 16], f32, isOutput=False)
    od = nc.declare_dram_parameter("od", [N_TILES * 8 * TILE_F], f32,
                                   isOutput=True)

    pk3 = pk[:].rearrange("(t p cf) -> t p cf", p=P, cf=3 * TILE_F)
    wm2 = wm[:].rearrange("(p j) -> p j", j=16)
    od3 = od[:].rearrange("(t r f) -> t r f", r=8, f=TILE_F)

    t_in = [nc.alloc_sbuf_tensor(f"t_in{i}", [P, 3 * TILE_F], f32).ap()
            for i in range(3)]
    t_pr = [nc.alloc_sbuf_tensor(f"t_pr{i}", [P, TILE_F], bf16).ap()
            for i in range(3)]
    t_lp = [nc.alloc_sbuf_tensor(f"t_lp{i}", [P, TILE_F], bf16).ap()
            for i in range(2)]
    t_lq = [nc.alloc_sbuf_tensor(f"t_lq{i}", [P, TILE_F], bf16).ap()
            for i in range(2)]
    t_d = [nc.alloc_sbuf_tensor(f"t_d{i}", [P, TILE_F], bf16).ap()
           for i in range(2)]
    t_e1 = [nc.alloc_sbuf_tensor(f"t_e1{i}", [P, TILE_F], bf16).ap()
            for i in range(2)]
    t_sb = [nc.alloc_sbuf_tensor(f"t_sb{i}", [8, TILE_F], f32).ap()
            for i in range(2)]
    w32 = nc.alloc_sbuf_tensor("w32", [P, 16], f32).ap()
    w16 = nc.alloc_sbuf_tensor("w16", [P, 16], bf16).ap()

    ps = [nc.alloc_psum_tensor(f"ps{i}", [8, TILE_F], f32).ap()
          for i in range(2)]

    s_in = [nc.alloc_semaphore(f"s_in{i}") for i in range(3)]   # +16 per load
    s_o = [nc.alloc_semaphore(f"s_o{i}") for i in range(2)]     # +16 per store
    s_wld = nc.alloc_semaphore("s_wld")                         # +16 W load
    s_dve = nc.alloc_semaphore("s_dve")  # +1 per DVE op
    s_act = nc.alloc_semaphore("s_act")  # +1 per ACT op
    s_pe = nc.alloc_semaphore("s_pe")    # +1 per tile (4th matmul)

    # Semaphore indices.
    # DVE: w16cvt=1, then per tile pr/d/e1.
    dve_i = {}
    n = 1
    for t in range(N_TILES):
        for o in ("pr", "d", "e1"):
            n += 1
            dve_i[(o, t)] = n
    # ACT: lq(t), lp(t), then cp(t-1) skewed one tile late.
    act_i = {}
    n = 0
    act_order = []
    for t in range(N_TILES):
        act_order.append(("lq", t))
        act_order.append(("lp", t))
        if t >= 1:
            act_order.append(("cp", t - 1))
    act_order.append(("cp", N_TILES - 1))
    for o in act_order:
        n += 1
        act_i[o] = n

    def in_wait(t):
        return (s_in[t % 3], 16 * (t // 3 + 1))

    with nc.Block() as block:

        @block.gpsimd
        def _(g):
            g.dma_start(w32, wm2).then_inc(s_wld, 16)
            for t in range(N_TILES):
                ins = g.dma_start(t_in[t % 3], pk3[t, :, :])
                if t >= 3:
                    # input slot free once lp(t-3) done (covers lq + DVE pr)
                    ins._wait_ge(s_act, act_i[("lp", t - 3)])
                ins.then_inc(s_in[t % 3], 16)

        @block.vector
        def _(v):
            v.tensor_copy(w16, w32)._wait_ge(s_wld, 16).then_inc(s_dve, 1)
            for t in range(N_TILES):
                s3, s2 = t % 3, t % 2
                yt_c = t_in[s3][:, TILE_F:2 * TILE_F]
                w_c = t_in[s3][:, 2 * TILE_F:3 * TILE_F]
                v.tensor_mul(t_pr[s3], yt_c, w_c) \
                    ._wait_ge(*in_wait(t)).then_inc(s_dve, 1)
                v.tensor_sub(t_d[s2], t_lp[s2], t_lq[s2]) \
                    ._wait_ge(s_act, act_i[("lp", t)]).then_inc(s_dve, 1)
                # same-engine RAW on d(t) needs an explicit retire wait
                v.wait_ge(s_dve, dve_i[("d", t)])
                ins = v.tensor_mul(t_e1[s2], t_pr[s3], t_d[s2])
                if t >= 2:
                    # e1/pr16 slots free once mm(t-2) retired
                    ins._wait_ge(s_pe, t - 1)
                ins.then_inc(s_dve, 1)

        @block.scalar
        def _(s):
            s.wait_ge(s_cst, 2)

            def emit_cp(tt):
                if tt >= 2:
                    s.wait_ge(s_o[tt % 2], 16 * ((tt - 2) // 2 + 1))
                s.activation(t_sb[tt % 2], ps[tt % 2], Copy) \
                    ._wait_ge(s_pe, tt + 1).then_inc(s_act, 1)

            for t in range(N_TILES):
                s3, s2 = t % 3, t % 2
                yp_c = t_in[s3][:, 0:TILE_F]
                s.activation(t_lq[s2], yp_c, Ln, bias=EPS) \
                    ._wait_ge(*in_wait(t)).then_inc(s_act, 1)
                s.activation(t_lp[s2], t_pr[s3], Ln, bias=TINY) \
                    ._wait_ge(s_dve, dve_i[("pr", t)]).then_inc(s_act, 1)
                if t >= 1:
                    emit_cp(t - 1)
            emit_cp(N_TILES - 1)

        @block.tensor
        def _(p):
            w8a = w16[:, 0:8]
            w8b = w16[:, 8:16]
            # ldweights of the first matmul precedes its inline wait; order
            # it after the w16 conversion explicitly
            p.wait_ge(s_dve, 1)
            for t in range(N_TILES):
                s3, s2 = t % 3, t % 2
                if t >= 2:
                    # psum slot free once cp(t-2) retired
                    p.wait_ge(s_act, act_i[("cp", t - 2)])
                p.matmul(ps[s2][:, 0:HALF], w8a, t_e1[s2][:, 0:HALF],
                         start=True, stop=False) \
                    ._wait_ge(s_dve, dve_i[("e1", t)])
                p.matmul(ps[s2][:, 0:HALF], w8b, t_e1[s2][:, HALF:TILE_F],
                         start=False, stop=True)
                p.matmul(ps[s2][:, HALF:TILE_F], w8a, t_pr[s3][:, 0:HALF],
                         start=True, stop=False)
                p.matmul(ps[s2][:, HALF:TILE_F], w8b, t_pr[s3][:, HALF:TILE_F],
                         start=False, stop=True).then_inc(s_pe, 1)

        @block.sync
        def _(sp):
            for t in range(N_TILES):
                s2 = t % 2
                sp.dma_start(od3[t, :, :], t_sb[s2]) \
                    ._wait_ge(s_act, act_i[("cp", t)]).then_inc(s_o[s2], 16)
            sp.wait_ge(s_o[0], 16 * (N_TILES // 2))
            sp.wait_ge(s_o[1], 16 * (N_TILES // 2))

    _check_one_wait(nc)
    return nc


def _get_program():
    if "nc" not in _CACHE:
        _CACHE["nc"] = _build_program()
    return _CACHE["nc"]


def _pack_inputs(yp, yt, w):
    """[N_TOTAL] f32 x3 -> per-core packed [t, p, {yp,yt,w}, f] arrays."""
    def to_tiles(x):
        # element i_local = t*TILE_ELEMS + f*P + p  ->  [core, t, p, f]
        return x.reshape(N_CORES, N_TILES, TILE_F, P).transpose(0, 1, 3, 2)

    pk = np.stack([to_tiles(yp), to_tiles(yt), to_tiles(w)], axis=3)
    return np.ascontiguousarray(pk).reshape(N_CORES, -1)


_WMAT = None


def _wmat():
    global _WMAT
    if _WMAT is None:
        wmat = np.zeros((P, 16), dtype=np.float32)
        for j in range(JBLK):
            wmat[BLK * j:BLK * (j + 1), j] = 1.0        # half 0 -> rows 0:4
            wmat[BLK * j:BLK * (j + 1), 12 + j] = 1.0   # half 1 -> rows 4:8
        _WMAT = wmat.reshape(-1)
    return _WMAT


def _run_device(yp, yt, w, trace=False):
    from concourse.bass_utils import run_bass_kernel_spmd

    nc = _get_program()
    pk = _pack_inputs(yp, yt, w)
    wmat = _wmat()
    in_maps = [{"pk": pk[k], "wm": wmat} for k in range(N_CORES)]
    res = run_bass_kernel_spmd(nc, in_maps, list(range(N_CORES)), trace=trace)

    bs1_parts, bs2_parts = [], []
    for r in res.results:
        dev = r["od"].reshape(N_TILES, 8, TILE_F)
        # psum row r = quad j + 4*half h; block id = t*4096 + (512h+f')*4 + j
        b1 = dev[:, :, 0:HALF].reshape(N_TILES, 2, JBLK, HALF)
        b2 = dev[:, :, HALF:TILE_F].reshape(N_TILES, 2, JBLK, HALF)
        bs1_parts.append(b1.transpose(0, 1, 3, 2).reshape(-1))
        bs2_parts.append(b2.transpose(0, 1, 3, 2).reshape(-1))
    return np.concatenate(bs1_parts), np.concatenate(bs2_parts), res


def kernel(y_pred, y_true, weight, segment_ptr, _trace=False):
    yp = np.ascontiguousarray(np.asarray(y_pred), dtype=np.float32).reshape(-1)
    yt = np.ascontiguousarray(np.asarray(y_true), dtype=np.float32).reshape(-1)
    w = np.ascontiguousarray(np.asarray(weight), dtype=np.float32).reshape(-1)
    ptr = np.asarray(segment_ptr).astype(np.int64).reshape(-1)
    n = yp.shape[0]
    G = ptr.shape[0] - 1
    assert n == N_TOTAL, f"kernel compiled for N={N_TOTAL}, got {n}"

    bs1, bs2, res = _run_device(yp, yt, w, trace=_trace)
    _CACHE["last_res"] = res

    # ---- host assembly in fp64 ----
    pre1 = np.empty(bs1.shape[0] + 1)
    pre1[0] = 0.0
    np.cumsum(bs1, dtype=np.float64, out=pre1[1:])
    pre2 = np.empty(bs2.shape[0] + 1)
    pre2[0] = 0.0
    np.cumsum(bs2, dtype=np.float64, out=pre2[1:])

    # clip ptr defensively to [0, n] (reference guarantees this range)
    ptrc = np.clip(ptr, 0, n)
    b_idx = ptrc // BLK
    r = ptrc - b_idx * BLK  # offset within block
    # fp64 partial sums over [ptr - r, ptr) for boundaries not block-aligned
    seg_off = np.concatenate([[0], np.cumsum(r)])
    tot = int(seg_off[-1])
    part1 = np.zeros(ptrc.shape[0])
    part2 = np.zeros(ptrc.shape[0])
    if tot > 0:
        idx = np.repeat(ptrc - r, r) + (np.arange(tot) - np.repeat(seg_off[:-1], r))
        pr_h = yt[idx].astype(np.float64) * w[idx].astype(np.float64)
        e1_h = pr_h * (np.log(pr_h + TINY) - np.log(yp[idx].astype(np.float64) + EPS))
        nz = r > 0
        red_idx = np.minimum(seg_off[:-1][nz], tot - 1).astype(np.int64)
        part1[nz] = np.add.reduceat(e1_h, red_idx)
        part2[nz] = np.add.reduceat(pr_h, red_idx)

    C1 = pre1[b_idx] + part1
    C2 = pre2[b_idx] + part2
    A = np.diff(C1)
    Bg = np.diff(C2)
    S = np.maximum(Bg, EPS)
    total = np.sum((A - Bg * np.log(S)) / S) / max(G, 1)
    return np.float32(total)


# revision 14
# speedup vs baseline: 1.9460x; 1.0261x over previous
"""Graphwise KL loss (segment_reduce) on 8 trn2 NeuronCores.

Strategy:
  Device (O(N) memory-bound work, data-parallel over 8 cores; each core
  streams a contiguous 1/8 slice, inputs host-packed to bf16):
    pr = y_true * weight                      (DVE bf16 2x)
    d  = ln(pr + 1e-37) - ln(y_pred + 1e-8)   (ACT Ln x2 -> bf16, DVE sub)
    e1 = pr * d                               (DVE bf16 2x)
    32-element block sums of e1 and pr        (PE matmul, block-diag ones)
  Host (O(num_graphs) metadata assembly, fp64): reconstruct per-segment
  sums A_g (e1) and B_g (pr) from device block sums + fp64 partial sums
  at segment boundaries; with S_g = max(B_g, EPS):
      total = mean_g (A_g - B_g * ln(S_g)) / S_g

  Inputs are packed into ONE interleaved bf16 DRAM tensor per core laid
  out [tile, partition, {yp,yt,w}, col] with element i = 128*col + part
  inside a tile, so each 32-element block sits in one partition quad and
  PE reduces blocks via matmul.  The stationary is a [128,16] pair of
  block-diagonal ones matrices (cols 0:8 -> psum rows 0:4 for the first
  half of a chunk's columns, cols 8:16 -> rows 4:8 for the second half)
  so a whole chunk accumulates into one [8, w] PSUM region: e1 sums in
  cols 0:w/2, pr sums in w/2:w.  PSUM is evacuated split: ACT copies the
  e1 half, DVE the pr half, both skewed one chunk late so they never
  block the next chunk's front end.  One load DMA + one store DMA per
  chunk; 3-deep input buffers keep the 16 SDMA engines streaming.

  The work is cut into chunks: 7 full tiles of 1024 columns plus 4
  quarter tiles of 256 at the end, so the serial drain chain after the
  last load is short.

  Raw Bass (no Tile): every op carries at most ONE inline sync wait
  (walrus cap); extra deps use standalone wait_ge instructions.
    POOL: load DMAs    DVE: pr/d/e1 + pr-psum evac
    ACT:  Ln x2 + e1-psum evac    PE: 4 matmuls/chunk    SP: store DMAs
"""

import numpy as np

N_TOTAL = 8388608
N_CORES = 8
N_LOCAL = N_TOTAL // N_CORES      # 1048576
P = 128
TILE_F = 1024                     # columns per full tile
N_TILES = N_LOCAL // (P * TILE_F)  # 8
BLK = 32
JBLK = P // BLK                   # 4 blocks per column
N_BLOCKS_LOCAL = N_LOCAL // BLK   # 32768
EPS = 1e-8
TINY = 1e-37

# chunk list: (tile, col0, width) — last tile split into quarters
CHUNKS = [(t, 0, TILE_F) for t in range(N_TILES - 1)]
CHUNKS += [(N_TILES - 1, c0, TILE_F // 4)
           for c0 in range(0, TILE_F, TILE_F // 4)]
NC_CH = len(CHUNKS)               # 11
PRC = TILE_F // 2                 # psum col of the pr group (bank-aligned)

_CACHE = {}


def _check_one_wait(nc):
    """Assert no non-EventSemaphore instruction carries more than one wait."""
    bad = []
    for f in nc.m.functions:
        for bb in f.blocks:
            for inst in bb.instructions:
                si = inst.sync_info
                if si and si.on_wait and len(si.on_wait) > 1:
                    if "EventSem" not in type(inst).__name__:
                        bad.append((type(inst).__name__, inst.name, len(si.on_wait)))
    assert not bad, f"multi-wait instructions remain: {bad}"


def _build_program():
    import concourse.bass as bass
    import concourse.mybir as mybir

    f32 = mybir.dt.float32
    bf16 = mybir.dt.bfloat16
    Ln = mybir.ActivationFunctionType.Ln
    Copy = mybir.ActivationFunctionType.Copy

    nc = bass.Bass()

    # Const APs for the Ln biases (activation() looks these up by value).
    # Emitted on the otherwise-idle DVE so POOL can start load DMAs at once.
    s_cst = nc.alloc_semaphore("s_cst")
    for val in (TINY, EPS):
        ct = nc.alloc_sbuf_tensor(f"const-f32-{val}", [128, 1], f32)
        nc.vector.memset(ct.ap(), val).then_inc(s_cst, 1)
        nc.const_aps.aps[(f32, val)] = ct.ap()

    pk = nc.declare_dram_parameter("pk", [3 * N_LOCAL], bf16, isOutput=False)
    wm = nc.declare_dram_parameter("wm", [P * 16], f32, isOutput=False)
    od = nc.declare_dram_parameter("od", [NC_CH * 8 * TILE_F], f32,
                                   isOutput=True)

    pk4 = pk[:].rearrange("(t p c f) -> t p c f", p=P, c=3, f=TILE_F)
    wm2 = wm[:].rearrange("(p j) -> p j", j=16)
    od3 = od[:].rearrange("(i r f) -> i r f", r=8, f=TILE_F)

    t_in = [nc.alloc_sbuf_tensor(f"t_in{i}", [P, 3, TILE_F], bf16).ap()
            for i in range(3)]
    t_pr = [nc.alloc_sbuf_tensor(f"t_pr{i}", [P, TILE_F], bf16).ap()
            for i in range(3)]
    t_lp = [nc.alloc_sbuf_tensor(f"t_lp{i}", [P, TILE_F], bf16).ap()
            for i in range(2)]
    t_lq = [nc.alloc_sbuf_tensor(f"t_lq{i}", [P, TILE_F], bf16).ap()
            for i in range(2)]
    t_d = [nc.alloc_sbuf_tensor(f"t_d{i}", [P, TILE_F], bf16).ap()
           for i in range(2)]
    t_e1 = [nc.alloc_sbuf_tensor(f"t_e1{i}", [P, TILE_F], bf16).ap()
            for i in range(2)]
    t_sb = [nc.alloc_sbuf_tensor(f"t_sb{i}", [8, TILE_F], f32).ap()
            for i in range(2)]
    w32 = nc.alloc_sbuf_tensor("w32", [P, 16], f32).ap()
    w16 = nc.alloc_sbuf_tensor("w16", [P, 16], bf16).ap()

    ps = [nc.alloc_psum_tensor(f"ps{i}", [8, TILE_F], f32).ap()
          for i in range(2)]

    s_in = [nc.alloc_semaphore(f"s_in{i}") for i in range(3)]   # +16 per load
    s_o = [nc.alloc_semaphore(f"s_o{i}") for i in range(2)]     # +16 per store
    s_wld = nc.alloc_semaphore("s_wld")                         # +16 W load
    s_dve = nc.alloc_semaphore("s_dve")  # +1 per DVE op
    s_act = nc.alloc_semaphore("s_act")  # +1 per ACT op
    s_pe = nc.alloc_semaphore("s_pe")    # +1 per chunk (4th matmul)

    # Op indices along each engine's in-order stream.
    dve_i = {}
    n = 1                                 # w16 convert = 1
    for c in range(NC_CH):
        for o in ("pr", "d", "e1"):
            n += 1
            dve_i[(o, c)] = n
        if c >= 1:
            n += 1
            dve_i[("cpb", c - 1)] = n
    n += 1
    dve_i[("cpb", NC_CH - 1)] = n

    act_i = {}
    n = 0
    for c in range(NC_CH):
        for o in ("lq", "lp"):
            n += 1
            act_i[(o, c)] = n
        if c >= 1:
            n += 1
            act_i[("cpa", c - 1)] = n
    n += 1
    act_i[("cpa", NC_CH - 1)] = n

    def in_wait(c):
        return (s_in[c % 3], 16 * (c // 3 + 1))

    def o_wait(c):
        # store of chunk c retired
        return (s_o[c % 2], 16 * (c // 2 + 1))

    with nc.Block() as block:

        @block.gpsimd
        def _(g):
            g.dma_start(w32, wm2).then_inc(s_wld, 16)
            for c, (t, c0, w) in enumerate(CHUNKS):
                ins = g.dma_start(t_in[c % 3][:, :, 0:w],
                                  pk4[t, :, :, c0:c0 + w])
                if c >= 3:
                    # input slot free once lp(c-3) done (covers lq + DVE pr)
                    ins._wait_ge(s_act, act_i[("lp", c - 3)])
                ins.then_inc(s_in[c % 3], 16)

        @block.vector
        def _(v):
            v.tensor_copy(w16, w32)._wait_ge(s_wld, 16).then_inc(s_dve, 1)

            def emit_cpb(cc):
                _, _, w = CHUNKS[cc]
                if cc >= 2:
                    v.wait_ge(*o_wait(cc - 2))
                v.tensor_copy(t_sb[cc % 2][:, PRC:PRC + w // 2],
                              ps[cc % 2][:, PRC:PRC + w // 2]) \
                    ._wait_ge(s_pe, cc + 1).then_inc(s_dve, 1)

            for c, (t, c0, w) in enumerate(CHUNKS):
                s3, s2 = c % 3, c % 2
                v.tensor_mul(t_pr[s3][:, 0:w], t_in[s3][:, 1, 0:w],
                             t_in[s3][:, 2, 0:w]) \
                    ._wait_ge(*in_wait(c)).then_inc(s_dve, 1)
                v.tensor_sub(t_d[s2][:, 0:w], t_lp[s2][:, 0:w],
                             t_lq[s2][:, 0:w]) \
                    ._wait_ge(s_act, act_i[("lp", c)]).then_inc(s_dve, 1)
                # same-engine RAW on d(c) needs an explicit retire wait
                v.wait_ge(s_dve, dve_i[("d", c)])
                ins = v.tensor_mul(t_e1[s2][:, 0:w], t_pr[s3][:, 0:w],
                                   t_d[s2][:, 0:w])
                if c >= 2:
                    ins._wait_ge(s_pe, c - 1)   # mm(c-2) done: slots free
                ins.then_inc(s_dve, 1)
                if c >= 1:
                    emit_cpb(c - 1)
            emit_cpb(NC_CH - 1)

        @block.scalar
        def _(s):
            s.wait_ge(s_cst, 2)

            def emit_cpa(cc):
                _, _, w = CHUNKS[cc]
                if cc >= 2:
                    s.wait_ge(*o_wait(cc - 2))
                s.activation(t_sb[cc % 2][:, 0:w // 2],
                             ps[cc % 2][:, 0:w // 2], Copy) \
                    ._wait_ge(s_pe, cc + 1).then_inc(s_act, 1)

            for c, (t, c0, w) in enumerate(CHUNKS):
                s3, s2 = c % 3, c % 2
                s.activation(t_lq[s2][:, 0:w], t_in[s3][:, 0, 0:w], Ln,
                             bias=EPS) \
                    ._wait_ge(*in_wait(c)).then_inc(s_act, 1)
                s.activation(t_lp[s2][:, 0:w], t_pr[s3][:, 0:w], Ln,
                             bias=TINY) \
                    ._wait_ge(s_dve, dve_i[("pr", c)]).then_inc(s_act, 1)
                if c >= 1:
                    emit_cpa(c - 1)
            emit_cpa(NC_CH - 1)

        @block.tensor
        def _(p):
            w8a = w16[:, 0:8]
            w8b = w16[:, 8:16]
            # ldweights of the first matmul precedes its inline wait; order
            # it after the w16 conversion explicitly
            p.wait_ge(s_dve, 1)
            for c, (t, c0, w) in enumerate(CHUNKS):
                s3, s2 = c % 3, c % 2
                h = w // 2
                if c >= 2:
                    # psum slot free once cpa(c-2) retired (cpb via s_dve)
                    p.wait_ge(s_act, act_i[("cpa", c - 2)])
                p.matmul(ps[s2][:, 0:h], w8a, t_e1[s2][:, 0:h],
                         start=True, stop=False) \
                    ._wait_ge(s_dve, dve_i[("e1", c)])
                p.matmul(ps[s2][:, 0:h], w8b, t_e1[s2][:, h:w],
                         start=False, stop=True)
                p.matmul(ps[s2][:, PRC:PRC + h], w8a, t_pr[s3][:, 0:h],
                         start=True, stop=False)
                p.matmul(ps[s2][:, PRC:PRC + h], w8b, t_pr[s3][:, h:w],
                         start=False, stop=True).then_inc(s_pe, 1)

        @block.sync
        def _(sp):
            for c, (t, c0, w) in enumerate(CHUNKS):
                s2 = c % 2
                h = w // 2
                src = t_sb[s2].rearrange("r (g f) -> r g f", g=2)[:, :, 0:h]
                dst = od3[c, :, 0:w].rearrange("r (g f) -> r g f", g=2)
                sp.wait_ge(s_dve, dve_i[("cpb", c)])
                sp.dma_start(dst, src) \
                    ._wait_ge(s_act, act_i[("cpa", c)]).then_inc(s_o[s2], 16)
            sp.wait_ge(s_o[0], 16 * ((NC_CH + 1) // 2))
            sp.wait_ge(s_o[1], 16 * (NC_CH // 2))

    _check_one_wait(nc)
    return nc


def _get_program():
    if "nc" not in _CACHE:
        _CACHE["nc"] = _build_program()
    return _CACHE["nc"]


def _pack_inputs(yp, yt, w):
    """[N_TOTAL] f32 x3 -> per-core packed bf16 [t, p, {yp,yt,w}, f]."""
    import ml_dtypes

    def to_tiles(x):
        # element i_local = t*P*TILE_F + f*P + p  ->  [core, t, p, f]
        return x.reshape(N_CORES, N_TILES, TILE_F, P).transpose(0, 1, 3, 2)

    pk = np.stack([to_tiles(yp), to_tiles(yt), to_tiles(w)], axis=3)
    pk = np.ascontiguousarray(pk).astype(ml_dtypes.bfloat16)
    return pk.reshape(N_CORES, -1)


_WMAT = None


def _wmat():
    global _WMAT
    if _WMAT is None:
        wmat = np.zeros((P, 16), dtype=np.float32)
        for j in range(JBLK):
            wmat[BLK * j:BLK * (j + 1), j] = 1.0        # half 0 -> rows 0:4
            wmat[BLK * j:BLK * (j + 1), 12 + j] = 1.0   # half 1 -> rows 4:8
        _WMAT = wmat.reshape(-1)
    return _WMAT


def _run_device(yp, yt, w, trace=False):
    from concourse.bass_utils import run_bass_kernel_spmd

    nc = _get_program()
    pk = _pack_inputs(yp, yt, w)
    wmat = _wmat()
    in_maps = [{"pk": pk[k], "wm": wmat} for k in range(N_CORES)]
    res = run_bass_kernel_spmd(nc, in_maps, list(range(N_CORES)), trace=trace)

    bs1_parts, bs2_parts = [], []
    for r in res.results:
        dev = r["od"].reshape(NC_CH, 8, TILE_F)
        bs1 = np.empty(N_BLOCKS_LOCAL, dtype=np.float64)
        bs2 = np.empty(N_BLOCKS_LOCAL, dtype=np.float64)
        for c, (t, c0, w) in enumerate(CHUNKS):
            h = w // 2
            # psum row r = quad j + 4*half; block = t*4096 + (c0+h*half+f')*4+j
            base = t * (TILE_F * JBLK) + c0 * JBLK
            nblk = w * JBLK
            b1 = dev[c, :, 0:h].reshape(2, JBLK, h)
            b2 = dev[c, :, h:w].reshape(2, JBLK, h)
            bs1[base:base + nblk] = b1.transpose(0, 2, 1).reshape(-1)
            bs2[base:base + nblk] = b2.transpose(0, 2, 1).reshape(-1)
        bs1_parts.append(bs1)
        bs2_parts.append(bs2)
    return np.concatenate(bs1_parts), np.concatenate(bs2_parts), res


def kernel(y_pred, y_true, weight, segment_ptr, _trace=False):
    yp = np.ascontiguousarray(np.asarray(y_pred), dtype=np.float32).reshape(-1)
    yt = np.ascontiguousarray(np.asarray(y_true), dtype=np.float32).reshape(-1)
    w = np.ascontiguousarray(np.asarray(weight), dtype=np.float32).reshape(-1)
    ptr = np.asarray(segment_ptr).astype(np.int64).reshape(-1)
    n = yp.shape[0]
    G = ptr.shape[0] - 1
    assert n == N_TOTAL, f"kernel compiled for N={N_TOTAL}, got {n}"

    bs1, bs2, res = _run_device(yp, yt, w, trace=_trace)
    _CACHE["last_res"] = res

    # ---- host assembly in fp64 ----
    pre1 = np.empty(bs1.shape[0] + 1)
    pre1[0] = 0.0
    np.cumsum(bs1, dtype=np.float64, out=pre1[1:])
    pre2 = np.empty(bs2.shape[0] + 1)
    pre2[0] = 0.0
    np.cumsum(bs2, dtype=np.float64, out=pre2[1:])

    # clip ptr defensively to [0, n] (reference guarantees this range)
    ptrc = np.clip(ptr, 0, n)
    b_idx = ptrc // BLK
    r = ptrc - b_idx * BLK  # offset within block
    # fp64 partial sums over [ptr - r, ptr) for boundaries not block-aligned
    seg_off = np.concatenate([[0], np.cumsum(r)])
    tot = int(seg_off[-1])
    part1 = np.zeros(ptrc.shape[0])
    part2 = np.zeros(ptrc.shape[0])
    if tot > 0:
        idx = np.repeat(ptrc - r, r) + (np.arange(tot) - np.repeat(seg_off[:-1], r))
        pr_h = yt[idx].astype(np.float64) * w[idx].astype(np.float64)
        e1_h = pr_h * (np.log(pr_h + TINY) - np.log(yp[idx].astype(np.float64) + EPS))
        nz = r > 0
        red_idx = np.minimum(seg_off[:-1][nz], tot - 1).astype(np.int64)
        part1[nz] = np.add.reduceat(e1_h, red_idx)
        part2[nz] = np.add.reduceat(pr_h, red_idx)

    C1 = pre1[b_idx] + part1
    C2 = pre2[b_idx] + part2
    A = np.diff(C1)
    Bg = np.diff(C2)
    S = np.maximum(Bg, EPS)
    total = np.sum((A - Bg * np.log(S)) / S) / max(G, 1)
    return np.float32(total)
